# revision 3
# baseline (speedup 1.0000x reference)
"""GQA multi-head attention (B=2, S=2048, D=2048, 32 q-heads / 8 kv-heads)
on 8 Trainium2 NeuronCores.

Sharding: tensor-parallel over kv-head groups. Core c owns kv head c and its
4 query heads: Wq column-shard [2048, 256], Wk/Wv column-shard [2048, 64],
Wo row-shard [256, 2048]. Each core computes a full-shape partial output
(its heads' contribution through Wo); the host sums the 8 partials.

Per-core dataflow (all matmuls bf16 operands, fp32 PSUM accumulate):
  QT  [256, B*S] = Wq_c^T @ q^T          (q^T fed from host)
  KT  [64,  B*S] = Wk_c^T @ k^T          (duplicated to partitions 0:64 and
                                          64:128 so even/odd head scores can
                                          row-pack and run concurrently on PE)
  V   [B*S, 64]  = v rows @ Wv_c         (natural layout; +ones column)
  S^T [k, q] = KT-tile.T @ QT            (scores transposed: softmax axis on
                                          partitions -> no P transpose needed)
  expS^T = exp(S^T * 1/8)                (ACT, scale folded in; no max-sub:
                                          |scores/8| < ~6 so fp32 exp is safe)
  ctx^T+rowsum [65, q] = V_aug^T @ expS^T (ones column gives softmax denom)
  ctx_norm = ctx^T * (1/rowsum)          (reciprocal + K=1 ones-matmul
                                          broadcast along partitions)
  out_partial [B*S, 2048] = ctx_norm-tiles.T @ Wo_c
"""

from contextlib import ExitStack

import numpy as np
import ml_dtypes

import concourse.bass as bass
import concourse.mybir as mybir
import concourse.tile as tile
from concourse import bacc
from concourse.bass_utils import run_bass_kernel_spmd

BF16 = mybir.dt.bfloat16
F32 = mybir.dt.float32
AF = mybir.ActivationFunctionType

B, S, DM = 2, 2048, 2048
HKV, G, DH = 8, 4, 64
DQ = G * DH            # 256: per-core q-projection width
NC = 8
DT = DM // 128         # 16 contraction tiles
BS = B * S             # 4096
SCALE = 1.0 / 8.0      # 1/sqrt(64)

_cache = {}


def _emit(ctx, tc, qT, kT, vT, wq, wk, wv, wo, out):
    nc = tc.nc

    pp = ctx.enter_context(tc.tile_pool(name="persist", bufs=1))
    wq_sb = pp.tile([128, DT, DQ], BF16, tag="wq")
    wk_sb = pp.tile([128, DT, DH], BF16, tag="wk")
    wv_sb = pp.tile([128, DT, DH], BF16, tag="wv")
    wo_sb = pp.tile([128, 2, DM], BF16, tag="wo")
    qtp = pp.tile([128, 2, BS], BF16, tag="qtp")    # QT pairs [p, hp, b*S+s]
    ktd = pp.tile([128, BS], BF16, tag="ktd")       # KT duplicated both halves
    vsb = pp.tile([128, BS // 128, DH + 1], BF16, tag="vsb")  # V + ones col
    ctxT = pp.tile([128, 2, BS], BF16, tag="ctxT")  # normalized ctx^T pairs
    ones_sb = pp.tile([128, DH], F32, tag="ones")

    nc.sync.dma_start(wq_sb[:], wq.rearrange("(dt p) m -> p dt m", p=128))
    nc.sync.dma_start(wk_sb[:], wk.rearrange("(dt p) m -> p dt m", p=128))
    nc.sync.dma_start(wv_sb[:], wv.rearrange("(dt p) m -> p dt m", p=128))
    nc.sync.dma_start(wo_sb[:], wo.rearrange("(i p) d -> p i d", p=128))
    nc.gpsimd.memset(ones_sb[:], 1.0)
    nc.gpsimd.memset(vsb[:, :, DH], 1.0)

    stage = ctx.enter_context(tc.tile_pool(name="stage", bufs=2))
    expp = ctx.enter_context(tc.tile_pool(name="expp", bufs=4))
    smal = ctx.enter_context(tc.tile_pool(name="small", bufs=2))
    outp = ctx.enter_context(tc.tile_pool(name="outp", bufs=2))

    # ---------------- Phase A: projections ----------------
    with tc.tile_pool(name="psumA", bufs=1, space="PSUM") as psa:
        for b in range(B):
            bo = b * S
            for qc in range(4):
                so = qc * 512
                q_ch = stage.tile([128, DT, 512], BF16, tag="instage",
                                  name=f"q_ch_{b}_{qc}")
                nc.sync.dma_start(
                    q_ch[:],
                    qT[b].rearrange("(dt p) s -> p dt s", p=128)[:, :, so:so + 512])
                for m in range(2):
                    pq = psa.tile([128, 512], F32, tag="proj", bufs=4,
                                  name=f"pq_{b}_{qc}_{m}")
                    for dt in range(DT):
                        nc.tensor.matmul(
                            pq[:], wq_sb[:, dt, m * 128:(m + 1) * 128],
                            q_ch[:, dt, :], start=(dt == 0), stop=(dt == DT - 1))
                    nc.vector.tensor_copy(qtp[:, m, bo + so:bo + so + 512], pq[:])

                k_ch = stage.tile([128, DT, 512], BF16, tag="instage",
                                  name=f"k_ch_{b}_{qc}")
                nc.sync.dma_start(
                    k_ch[:],
                    kT[b].rearrange("(dt p) s -> p dt s", p=128)[:, :, so:so + 512])
                pk = psa.tile([128, 512], F32, tag="proj", bufs=4,
                              name=f"pk_{b}_{qc}")
                for dt in range(DT):
                    nc.tensor.matmul(
                        pk[0:DH, :], wk_sb[:, dt, :], k_ch[:, dt, :],
                        start=(dt == 0), stop=(dt == DT - 1))
                nc.vector.tensor_copy(ktd[0:DH, bo + so:bo + so + 512], pk[0:DH, :])
                nc.vector.tensor_copy(ktd[DH:128, bo + so:bo + so + 512], pk[0:DH, :])

                v_ch = stage.tile([128, DT, 512], BF16, tag="instage",
                                  name=f"v_ch_{b}_{qc}")
                nc.sync.dma_start(
                    v_ch[:],
                    vT[b].rearrange("(dt p) s -> p dt s", p=128)[:, :, so:so + 512])
                for ss in range(4):
                    pv = psa.tile([128, DH], F32, tag="vproj", bufs=2,
                                  name=f"pv_{b}_{qc}_{ss}")
                    for dt in range(DT):
                        nc.tensor.matmul(
                            pv[:], v_ch[:, dt, ss * 128:(ss + 1) * 128],
                            wv_sb[:, dt, :], start=(dt == 0), stop=(dt == DT - 1))
                    nc.vector.tensor_copy(
                        vsb[:, b * 16 + qc * 4 + ss, 0:DH], pv[:])

    # ------------- Phase B: attention per (batch, head-pair) -------------
    with tc.tile_pool(name="psumB", bufs=1, space="PSUM") as psb:
        for b in range(B):
            bo = b * S
            for hp in range(2):
                for qc in range(4):
                    qoff = bo + qc * 512
                    exps = [expp.tile([128, DT, 512], BF16, tag="exp",
                                      name=f"exp{j}_{b}_{hp}_{qc}")
                            for j in range(2)]
                    for kt2 in range(DT // 2):
                        pss = [psb.tile([128, 2, 512], F32, tag="sc", bufs=2,
                                        name=f"pss{j}_{b}_{hp}_{qc}_{kt2}")
                               for j in range(2)]
                        for j2 in range(2):
                            kt = 2 * kt2 + j2
                            koff = bo + kt * 128
                            for j in range(2):  # head j: rows j*64:(j+1)*64
                                lo, hi = j * DH, (j + 1) * DH
                                nc.tensor.matmul(
                                    pss[j][:, j2, :],
                                    ktd[lo:hi, koff:koff + 128],
                                    qtp[lo:hi, hp, qoff:qoff + 512])
                        for j in range(2):
                            nc.scalar.activation(
                                exps[j][:, 2 * kt2:2 * kt2 + 2, :], pss[j][:],
                                AF.Exp, scale=SCALE)
                    for j in range(2):
                        pc = psb.tile([128, 512], F32, tag="ctx", bufs=2,
                                      name=f"pc{j}_{b}_{hp}_{qc}")
                        for kt in range(DT):
                            nc.tensor.matmul(
                                pc[0:DH + 1, :], vsb[:, b * 16 + kt, :],
                                exps[j][:, kt, :],
                                start=(kt == 0), stop=(kt == DT - 1))
                        rc = smal.tile([128, 512], F32, tag="recip",
                                       name=f"rc{j}_{b}_{hp}_{qc}")
                        nc.vector.reciprocal(rc[DH:DH + 1, :], pc[DH:DH + 1, :])
                        pb = psb.tile([DH, 512], F32, tag="bc", bufs=2,
                                      name=f"pb{j}_{b}_{hp}_{qc}")
                        nc.tensor.matmul(
                            pb[:], ones_sb[DH:DH + 1, :], rc[DH:DH + 1, :])
                        bc_sb = smal.tile([DH, 512], F32, tag="bc_sb",
                                          name=f"bc_sb{j}_{b}_{hp}_{qc}")
                        nc.vector.tensor_copy(bc_sb[:], pb[:])
                        nc.vector.tensor_mul(
                            ctxT[j * DH:(j + 1) * DH, hp, qoff:qoff + 512],
                            pc[0:DH, :], bc_sb[:])

    # ---------------- Phase C: output projection ----------------
    with tc.tile_pool(name="psumC", bufs=1, space="PSUM") as psc:
        for st in range(BS // 128):
            ost = outp.tile([128, DM], F32, tag="ostage", name=f"ost_{st}")
            for ch in range(4):
                po = psc.tile([128, 512], F32, tag="wo", bufs=4,
                              name=f"po_{st}_{ch}")
                for i in range(2):
                    nc.tensor.matmul(
                        po[:], ctxT[:, i, st * 128:(st + 1) * 128],
                        wo_sb[:, i, ch * 512:(ch + 1) * 512],
                        start=(i == 0), stop=(i == 1))
                nc.vector.tensor_copy(ost[:, ch * 512:(ch + 1) * 512], po[:])
            nc.sync.dma_start(out[st * 128:(st + 1) * 128, :], ost[:])


def _build():
    nc = bacc.Bacc("TRN2", target_bir_lowering=False, debug=False, num_devices=NC)
    qT = nc.dram_tensor("qT", [B, DM, S], BF16, kind="ExternalInput")
    kT = nc.dram_tensor("kT", [B, DM, S], BF16, kind="ExternalInput")
    vT = nc.dram_tensor("vT", [B, DM, S], BF16, kind="ExternalInput")
    wq = nc.dram_tensor("wq", [DM, DQ], BF16, kind="ExternalInput")
    wk = nc.dram_tensor("wk", [DM, DH], BF16, kind="ExternalInput")
    wv = nc.dram_tensor("wv", [DM, DH], BF16, kind="ExternalInput")
    wo = nc.dram_tensor("wo", [DQ, DM], BF16, kind="ExternalInput")
    out = nc.dram_tensor("out", [BS, DM], F32, kind="ExternalOutput")
    with tile.TileContext(nc) as tc:
        with ExitStack() as ctx:
            _emit(ctx, tc, qT.ap(), kT.ap(), vT.ap(), wq.ap(), wk.ap(),
                  wv.ap(), wo.ap(), out.ap())
    nc.compile()
    return nc


def kernel(q, k, v, Wq, Wk, Wv, Wo, _trace=False, _tmpdir=None):
    q = np.asarray(q, dtype=np.float32)
    k = np.asarray(k, dtype=np.float32)
    v = np.asarray(v, dtype=np.float32)
    bf = ml_dtypes.bfloat16
    qTh = np.ascontiguousarray(q.astype(bf).transpose(0, 2, 1))
    kTh = np.ascontiguousarray(k.astype(bf).transpose(0, 2, 1))
    vTh = np.ascontiguousarray(v.astype(bf).transpose(0, 2, 1))
    Wqb = np.asarray(Wq, dtype=np.float32).astype(bf)
    Wkb = np.asarray(Wk, dtype=np.float32).astype(bf)
    Wvb = np.asarray(Wv, dtype=np.float32).astype(bf)
    Wob = np.asarray(Wo, dtype=np.float32).astype(bf)

    if "nc" not in _cache:
        _cache["nc"] = _build()
    nc = _cache["nc"]

    in_maps = []
    for c in range(NC):
        in_maps.append({
            "qT": qTh, "kT": kTh, "vT": vTh,
            "wq": np.ascontiguousarray(Wqb[:, c * DQ:(c + 1) * DQ]),
            "wk": np.ascontiguousarray(Wkb[:, c * DH:(c + 1) * DH]),
            "wv": np.ascontiguousarray(Wvb[:, c * DH:(c + 1) * DH]),
            "wo": np.ascontiguousarray(Wob[c * DQ:(c + 1) * DQ, :]),
        })
    kw = {}
    if _trace:
        kw = dict(trace=True, tmpdir=_tmpdir)
    res = run_bass_kernel_spmd(nc, in_maps, core_ids=list(range(NC)), **kw)
    out = res.results[0]["out"].astype(np.float32)
    for c in range(1, NC):
        out += res.results[c]["out"]
    if _trace:
        kernel.last_results = res
    return out.reshape(B, S, DM)


# revision 4
# speedup vs baseline: 26205.7417x; 26205.7417x over previous
"""GQA multi-head attention (B=2, S=2048, D=2048, 32 q-heads / 8 kv-heads)
on 8 Trainium2 NeuronCores.

Sharding: tensor-parallel over kv-head groups. Core c owns kv head c and its
4 query heads: Wq column-shard [2048, 256], Wk/Wv column-shard [2048, 64],
Wo row-shard [256, 2048]. Each core computes a full-shape partial output
(its heads' contribution through Wo); the host sums the 8 partials.

Per-core dataflow (all matmuls bf16 operands, fp32 PSUM accumulate):
  QT  [256, B*S] = Wq_c^T @ q^T          (q^T fed from host)
  KT  [64,  B*S] = Wk_c^T @ k^T          (duplicated to partitions 0:64 and
                                          64:128 so even/odd head scores can
                                          row-pack and run concurrently on PE)
  V   [B*S, 64]  = v rows @ Wv_c         (natural layout; +ones column)
  S^T [k, q] = KT-tile.T @ QT            (scores transposed: softmax axis on
                                          partitions -> no P transpose needed)
  expS^T = exp(S^T * 1/8)                (ACT, scale folded in; no max-sub:
                                          |scores/8| < ~6 so fp32 exp is safe)
  ctx^T+rowsum [65, q] = V_aug^T @ expS^T (ones column gives softmax denom)
  ctx_norm = ctx^T * (1/rowsum)          (DVE reciprocal into partition 0 +
                                          GPSIMD partition_broadcast)
  out_partial [B*S, 2048] = ctx_norm-tiles.T @ Wo_c
"""

from contextlib import ExitStack

import numpy as np
import ml_dtypes

import jax

try:
    jax.config.update("jax_compilation_cache_dir", "/tmp/jax_bass_cache")
    jax.config.update("jax_persistent_cache_min_compile_time_secs", 1.0)
except Exception:
    pass

from jax.sharding import Mesh, PartitionSpec, NamedSharding
from jax.experimental.shard_map import shard_map

import concourse.bass as bass
import concourse.mybir as mybir
import concourse.tile as tile
from concourse import bacc, bass2jax

BF16 = mybir.dt.bfloat16
F32 = mybir.dt.float32
AF = mybir.ActivationFunctionType

B, S, DM = 2, 2048, 2048
HKV, G, DH = 8, 4, 64
DQ = G * DH            # 256: per-core q-projection width
NC = 8
DT = DM // 128         # 16 contraction tiles
BS = B * S             # 4096
SCALE = 1.0 / 8.0      # 1/sqrt(64)

_cache = {}


def _emit(ctx, tc, qT, kT, vT, wq, wk, wv, wo, out):
    nc = tc.nc

    pp = ctx.enter_context(tc.tile_pool(name="persist", bufs=1))
    wq_sb = pp.tile([128, DT, DQ], BF16, tag="wq")
    wk_sb = pp.tile([128, DT, DH], BF16, tag="wk")
    wv_sb = pp.tile([128, DT, DH], BF16, tag="wv")
    wo_sb = pp.tile([128, 2, DM], BF16, tag="wo")
    qtp = pp.tile([128, 2, BS], BF16, tag="qtp")    # QT pairs [p, hp, b*S+s]
    ktd = pp.tile([128, BS], BF16, tag="ktd")       # KT duplicated both halves
    vsb = pp.tile([128, BS // 128, DH + 1], BF16, tag="vsb")  # V + ones col
    ctxT = pp.tile([128, 2, BS], BF16, tag="ctxT")  # normalized ctx^T pairs

    nc.sync.dma_start(wq_sb[:], wq.rearrange("(dt p) m -> p dt m", p=128))
    nc.sync.dma_start(wk_sb[:], wk.rearrange("(dt p) m -> p dt m", p=128))
    nc.sync.dma_start(wv_sb[:], wv.rearrange("(dt p) m -> p dt m", p=128))
    nc.sync.dma_start(wo_sb[:], wo.rearrange("(i p) d -> p i d", p=128))
    nc.gpsimd.memset(vsb[:, :, DH], 1.0)

    stage = ctx.enter_context(tc.tile_pool(name="stage", bufs=2))
    expp = ctx.enter_context(tc.tile_pool(name="expp", bufs=4))
    smal = ctx.enter_context(tc.tile_pool(name="small", bufs=2))
    outp = ctx.enter_context(tc.tile_pool(name="outp", bufs=2))
    psum = ctx.enter_context(tc.tile_pool(name="psum", bufs=1, space="PSUM"))

    # ---------------- Phase A: projections ----------------
    for b in range(B):
        bo = b * S
        for qc in range(4):
            so = qc * 512
            q_ch = stage.tile([128, DT, 512], BF16, tag="instage",
                              name=f"q_ch_{b}_{qc}")
            nc.sync.dma_start(
                q_ch[:],
                qT[b].rearrange("(dt p) s -> p dt s", p=128)[:, :, so:so + 512])
            for m in range(2):
                pq = psum.tile([128, 512], F32, tag="mm", bufs=3,
                               name=f"pq_{b}_{qc}_{m}")
                for dt in range(DT):
                    nc.tensor.matmul(
                        pq[:], wq_sb[:, dt, m * 128:(m + 1) * 128],
                        q_ch[:, dt, :], start=(dt == 0), stop=(dt == DT - 1))
                nc.vector.tensor_copy(qtp[:, m, bo + so:bo + so + 512], pq[:])

            k_ch = stage.tile([128, DT, 512], BF16, tag="instage",
                              name=f"k_ch_{b}_{qc}")
            nc.sync.dma_start(
                k_ch[:],
                kT[b].rearrange("(dt p) s -> p dt s", p=128)[:, :, so:so + 512])
            pk = psum.tile([128, 512], F32, tag="mm", bufs=3,
                           name=f"pk_{b}_{qc}")
            for dt in range(DT):
                nc.tensor.matmul(
                    pk[0:DH, :], wk_sb[:, dt, :], k_ch[:, dt, :],
                    start=(dt == 0), stop=(dt == DT - 1))
            nc.vector.tensor_copy(ktd[0:DH, bo + so:bo + so + 512], pk[0:DH, :])
            nc.vector.tensor_copy(ktd[DH:128, bo + so:bo + so + 512], pk[0:DH, :])

            v_ch = stage.tile([128, DT, 512], BF16, tag="instage",
                              name=f"v_ch_{b}_{qc}")
            nc.sync.dma_start(
                v_ch[:],
                vT[b].rearrange("(dt p) s -> p dt s", p=128)[:, :, so:so + 512])
            for ss in range(4):
                pv = psum.tile([128, DH], F32, tag="vp", bufs=1,
                               name=f"pv_{b}_{qc}_{ss}")
                for dt in range(DT):
                    nc.tensor.matmul(
                        pv[:], v_ch[:, dt, ss * 128:(ss + 1) * 128],
                        wv_sb[:, dt, :], start=(dt == 0), stop=(dt == DT - 1))
                nc.vector.tensor_copy(
                    vsb[:, b * 16 + qc * 4 + ss, 0:DH], pv[:])

    # ------------- Phase B: attention per (batch, head-pair) -------------
    for b in range(B):
        bo = b * S
        for hp in range(2):
            for qc in range(4):
                qoff = bo + qc * 512
                exps = [expp.tile([128, DT, 512], BF16, tag="exp",
                                  name=f"exp{j}_{b}_{hp}_{qc}")
                        for j in range(2)]
                for kt2 in range(DT // 2):
                    pss = [psum.tile([128, 2, 512], F32, tag="sc", bufs=2,
                                     name=f"pss{j}_{b}_{hp}_{qc}_{kt2}")
                           for j in range(2)]
                    for j2 in range(2):
                        kt = 2 * kt2 + j2
                        koff = bo + kt * 128
                        for j in range(2):  # head j: rows j*64:(j+1)*64
                            lo, hi = j * DH, (j + 1) * DH
                            nc.tensor.matmul(
                                pss[j][:, j2, :],
                                ktd[lo:hi, koff:koff + 128],
                                qtp[lo:hi, hp, qoff:qoff + 512])
                    for j in range(2):
                        nc.scalar.activation(
                            exps[j][:, 2 * kt2:2 * kt2 + 2, :], pss[j][:],
                            AF.Exp, scale=SCALE)
                for j in range(2):
                    pc = psum.tile([128, 512], F32, tag="mm", bufs=3,
                                   name=f"pc{j}_{b}_{hp}_{qc}")
                    for kt in range(DT):
                        nc.tensor.matmul(
                            pc[0:DH + 1, :], vsb[:, b * 16 + kt, :],
                            exps[j][:, kt, :],
                            start=(kt == 0), stop=(kt == DT - 1))
                    rc = smal.tile([128, 512], F32, tag="recip",
                                   name=f"rc{j}_{b}_{hp}_{qc}")
                    nc.vector.reciprocal(rc[0:1, :], pc[DH:DH + 1, :])
                    bc_sb = smal.tile([DH, 512], F32, tag="bc_sb",
                                      name=f"bc{j}_{b}_{hp}_{qc}")
                    nc.gpsimd.partition_broadcast(bc_sb[:], rc[0:1, :])
                    nc.vector.tensor_mul(
                        ctxT[j * DH:(j + 1) * DH, hp, qoff:qoff + 512],
                        pc[0:DH, :], bc_sb[:])

    # ---------------- Phase C: output projection ----------------
    for st in range(BS // 128):
        ost = outp.tile([128, DM], F32, tag="ostage", name=f"ost_{st}")
        for ch in range(4):
            po = psum.tile([128, 512], F32, tag="mm", bufs=3,
                           name=f"po_{st}_{ch}")
            for i in range(2):
                nc.tensor.matmul(
                    po[:], ctxT[:, i, st * 128:(st + 1) * 128],
                    wo_sb[:, i, ch * 512:(ch + 1) * 512],
                    start=(i == 0), stop=(i == 1))
            nc.vector.tensor_copy(ost[:, ch * 512:(ch + 1) * 512], po[:])
        nc.sync.dma_start(out[st * 128:(st + 1) * 128, :], ost[:])


def _build():
    nc = bacc.Bacc("TRN2", target_bir_lowering=False, debug=False, num_devices=NC)
    qT = nc.dram_tensor("qT", [B, DM, S], BF16, kind="ExternalInput")
    kT = nc.dram_tensor("kT", [B, DM, S], BF16, kind="ExternalInput")
    vT = nc.dram_tensor("vT", [B, DM, S], BF16, kind="ExternalInput")
    wq = nc.dram_tensor("wq", [DM, DQ], BF16, kind="ExternalInput")
    wk = nc.dram_tensor("wk", [DM, DH], BF16, kind="ExternalInput")
    wv = nc.dram_tensor("wv", [DM, DH], BF16, kind="ExternalInput")
    wo = nc.dram_tensor("wo", [DQ, DM], BF16, kind="ExternalInput")
    out = nc.dram_tensor("out", [BS, DM], F32, kind="ExternalOutput")
    with tile.TileContext(nc) as tc:
        with ExitStack() as ctx:
            _emit(ctx, tc, qT.ap(), kT.ap(), vT.ap(), wq.ap(), wk.ap(),
                  wv.ap(), wo.ap(), out.ap())
    nc.compile()
    return nc


def _make_runner(nc, n_cores=NC):
    """Build the sharded jit callable once; reuse across kernel() calls."""
    bass2jax.install_neuronx_cc_hook()
    partition_name = nc.partition_id_tensor.name if nc.partition_id_tensor else None
    in_names, out_names, out_avals, zero_outs = [], [], [], []
    for alloc in nc.m.functions[0].allocations:
        if not isinstance(alloc, mybir.MemoryLocationSet):
            continue
        name = alloc.memorylocations[0].name
        if alloc.kind == "ExternalInput":
            if name != partition_name:
                in_names.append(name)
        elif alloc.kind == "ExternalOutput":
            out_names.append(name)
            shape = tuple(alloc.tensor_shape)
            dtype = mybir.dt.np(alloc.dtype)
            out_avals.append(jax.core.ShapedArray(shape, dtype))
            zero_outs.append(np.zeros(shape, dtype))
    n_params = len(in_names)
    n_outs = len(out_avals)
    in_names_all = in_names + out_names
    if partition_name is not None:
        in_names_all.append(partition_name)
    donate = tuple(range(n_params, n_params + n_outs))

    def _body(*args):
        operands = list(args)
        if partition_name is not None:
            operands.append(bass2jax.partition_id_tensor())
        outs = bass2jax._bass_exec_p.bind(
            *operands,
            out_avals=tuple(out_avals),
            in_names=tuple(in_names_all),
            out_names=tuple(out_names),
            lowering_input_output_aliases=(),
            sim_require_finite=True,
            sim_require_nnan=True,
            nc=nc,
        )
        return tuple(outs)

    devices = jax.devices()[:n_cores]
    mesh = Mesh(np.asarray(devices), ("core",))
    in_specs = (PartitionSpec("core"),) * (n_params + n_outs)
    out_specs = (PartitionSpec("core"),) * len(out_names)
    sharded = jax.jit(
        shard_map(_body, mesh=mesh, in_specs=in_specs, out_specs=out_specs,
                  check_rep=False),
        donate_argnums=donate, keep_unused=True)
    sh = NamedSharding(mesh, PartitionSpec("core"))
    return sharded, in_names, out_names, zero_outs, sh


def _run(in_maps):
    if "nc" not in _cache:
        _cache["nc"] = _build()
    if "runner" not in _cache:
        _cache["runner"] = _make_runner(_cache["nc"])
    sharded, in_names, out_names, zero_outs, sh = _cache["runner"]
    n = NC
    concat_in = [
        jax.device_put(
            np.concatenate([np.asarray(in_maps[c][nm]) for c in range(n)], 0), sh)
        for nm in in_names
    ]
    zeros = [
        jax.device_put(np.zeros((n * z.shape[0], *z.shape[1:]), z.dtype), sh)
        for z in zero_outs
    ]
    outs = sharded(*concat_in, *zeros)
    i = out_names.index("out")
    arr = np.asarray(outs[i])           # [NC*BS, DM]
    return arr.reshape(n, BS, DM)


def kernel(q, k, v, Wq, Wk, Wv, Wo):
    q = np.asarray(q, dtype=np.float32)
    k = np.asarray(k, dtype=np.float32)
    v = np.asarray(v, dtype=np.float32)
    bf = ml_dtypes.bfloat16
    qTh = np.ascontiguousarray(q.astype(bf).transpose(0, 2, 1))
    kTh = np.ascontiguousarray(k.astype(bf).transpose(0, 2, 1))
    vTh = np.ascontiguousarray(v.astype(bf).transpose(0, 2, 1))
    Wqb = np.asarray(Wq, dtype=np.float32).astype(bf)
    Wkb = np.asarray(Wk, dtype=np.float32).astype(bf)
    Wvb = np.asarray(Wv, dtype=np.float32).astype(bf)
    Wob = np.asarray(Wo, dtype=np.float32).astype(bf)

    in_maps = []
    for c in range(NC):
        in_maps.append({
            "qT": qTh, "kT": kTh, "vT": vTh,
            "wq": np.ascontiguousarray(Wqb[:, c * DQ:(c + 1) * DQ]),
            "wk": np.ascontiguousarray(Wkb[:, c * DH:(c + 1) * DH]),
            "wv": np.ascontiguousarray(Wvb[:, c * DH:(c + 1) * DH]),
            "wo": np.ascontiguousarray(Wob[c * DQ:(c + 1) * DQ, :]),
        })
    partials = _run(in_maps)
    out = partials.astype(np.float32, copy=False).sum(axis=0)
    return out.reshape(B, S, DM)
